# revision 16
# baseline (speedup 1.0000x reference)
"""Multi-head differential attention on 8 Trainium2 NeuronCores.

Sharding: core c -> batch c//4, head-group c%4 (4 of 16 heads).
Per core: QKV projection for its heads, k-major attention (scores
transposed; softmax denominators come from a ones-row appended to V via
the AV matmul).  Softmax normalization is pipelined PER QUERY TILE: the
denominator row is reciprocal'd ([1,512] DVE), broadcast across 64
partitions (stride-0 DMA), and one DVE multiply writes the normalized
bf16 z straight into the gather payload; bn_stats runs on the bf16
slices.  Each head-pair's payload (z plus bitcast [mean,var] scalar
columns) is DMA'd per-qt into DRAM and AllGathered within the 4-core
batch group; pair 0's gather hides under pair 1's attention.  GN rstd
(sqrt) is computed on the CONSUMER side after the gathers land, so the
ACT engine does a single exp->sqrt table switch per kernel.  The
column-parallel out-projection accumulates gathered chunks in PSUM,
prefolding the pair-0 chunks while gather #2 is in flight; the GN
affine constant row rides the ACT copy bias.

Host side folds: lambda and softmax scale into Wq/bq; gn_w into Wo;
gb@Wo into bo.  x is pre-transposed per batch and cast to bf16.
"""

import numpy as np
import ml_dtypes

B, S, D, H, DH = 2, 2048, 1024, 16, 64
HPC = 4            # heads per core
CW = HPC * DH      # attention columns per core (256)
EPS = 1e-5
LAMBDA_INIT = 0.8
N_CORES = 8
SCC = 16           # scalar payload columns (bf16); 4 f32 used
NQT = 4            # query tiles of 512
QT = 512
NKT = 16           # key tiles of 128
NDC = 8            # d-chunks of 128

_cache = {}


def _build(with_collective=True, debug_taps=False):
    from contextlib import ExitStack
    import concourse.bass as bass
    from concourse import bacc
    import concourse.tile as tile
    import concourse.mybir as mybir

    f32 = mybir.dt.float32
    bf16 = mybir.dt.bfloat16
    AF = mybir.ActivationFunctionType
    ALU = mybir.AluOpType

    nc = bacc.Bacc("TRN2", target_bir_lowering=False, debug=False,
                   num_devices=N_CORES)

    xt_d = nc.dram_tensor("xt", [D, S], bf16, kind="ExternalInput")
    wq_d = nc.dram_tensor("wq", [D, CW], bf16, kind="ExternalInput")
    wk_d = nc.dram_tensor("wk", [D, CW], bf16, kind="ExternalInput")
    wv_d = nc.dram_tensor("wv", [D, CW], bf16, kind="ExternalInput")
    wo_d = nc.dram_tensor("wo", [D, CW], bf16, kind="ExternalInput")
    bq_d = nc.dram_tensor("bq2", [128, 2], f32, kind="ExternalInput")
    bk_d = nc.dram_tensor("bk2", [128, 2], f32, kind="ExternalInput")
    bv_d = nc.dram_tensor("bv", [64, HPC], f32, kind="ExternalInput")
    bvf_d = nc.dram_tensor("bvf", [D], f32, kind="ExternalInput")
    bo_d = nc.dram_tensor("bor", [1, CW], f32, kind="ExternalInput")
    y_d = nc.dram_tensor("y", [2, 128, S], f32, kind="ExternalOutput")

    if debug_taps:
        dbg_d = nc.dram_tensor("dbg", [128, 2 * NDC + CW], f32,
                               kind="ExternalOutput")
        dbgz_d = nc.dram_tensor("dbgz", [2, 128, S + SCC], f32,
                                kind="ExternalOutput")
    ag_in = [nc.dram_tensor(f"ag_in{t}", [128, S + SCC], bf16)
             for t in range(2)]
    ag_out = [nc.dram_tensor(f"ag_out{t}", [4, 128, S + SCC], bf16)
              for t in range(2)]

    with ExitStack() as ctx:
        tc = ctx.enter_context(tile.TileContext(nc))
        const = ctx.enter_context(tc.tile_pool(name="const", bufs=1))
        big = ctx.enter_context(tc.tile_pool(name="big", bufs=1))

        # ---- inputs, priority order: what the first matmuls need first ----
        wq_sb = const.tile([128, NDC, CW], bf16, tag="wq")
        nc.sync.dma_start(out=wq_sb[:, :, 0:128],
                          in_=wq_d[:, 0:128].rearrange("(c p) n -> p c n", p=128))
        bq_sb = const.tile([128, 2], f32, tag="bq")
        nc.sync.dma_start(out=bq_sb, in_=bq_d[:, :])
        pxt = ctx.enter_context(tc.tile_pool(name="pxt", bufs=1))
        xt_sb = [pxt.tile([128, S], bf16, tag=f"xt{c}", name=f"xt{c}")
                 for c in range(NDC)]
        nc.sync.dma_start(out=xt_sb[0], in_=xt_d[0:128, :])
        wk_sb = const.tile([128, NDC, CW], bf16, tag="wk")
        nc.sync.dma_start(out=wk_sb[:, :, 0:128],
                          in_=wk_d[:, 0:128].rearrange("(c p) n -> p c n", p=128))
        bk_sb = const.tile([128, 2], f32, tag="bk")
        nc.sync.dma_start(out=bk_sb, in_=bk_d[:, :])
        for c in range(1, 4):
            nc.sync.dma_start(out=xt_sb[c], in_=xt_d[c * 128:(c + 1) * 128, :])
        wv_sb = const.tile([128, NDC, CW], bf16, tag="wv")
        nc.sync.dma_start(out=wv_sb, in_=wv_d[:, :].rearrange("(c p) n -> p c n", p=128))
        for c in range(4, NDC):
            nc.sync.dma_start(out=xt_sb[c], in_=xt_d[c * 128:(c + 1) * 128, :])
        nc.sync.dma_start(out=wq_sb[:, :, 128:256],
                          in_=wq_d[:, 128:256].rearrange("(c p) n -> p c n", p=128))
        nc.sync.dma_start(out=wk_sb[:, :, 128:256],
                          in_=wk_d[:, 128:256].rearrange("(c p) n -> p c n", p=128))
        bv_sb = const.tile([64, HPC], f32, tag="bv")
        nc.sync.dma_start(out=bv_sb, in_=bv_d[:, :])
        wo_sb = const.tile([128, NDC, CW], bf16, tag="wo")
        nc.sync.dma_start(out=wo_sb, in_=wo_d[:, :].rearrange("(c p) n -> p c n", p=128))
        bo_sb = const.tile([1, CW], f32, tag="bo")
        nc.sync.dma_start(out=bo_sb, in_=bo_d[:, :])
        bvg_sb = const.tile([128, NDC], f32, tag="bvg")
        nc.sync.dma_start(out=bvg_sb, in_=bvf_d[:].rearrange("(c p) -> p c", p=128))

        onesc = const.tile([64, 1], f32, tag="onesc")
        nc.vector.memset(onesc, 1.0)
        ones128 = const.tile([128, QT], bf16, tag="ones128")
        nc.vector.memset(ones128, 1.0)
        eps_t = const.tile([1, 1], f32, tag="eps")
        nc.vector.memset(eps_t, EPS)

        qT_sb = big.tile([128, 2, S], bf16, tag="qT")   # pair t: head 2t rows 0:64
        kT_sb = big.tile([128, 2, S], bf16, tag="kT")
        v_sb = [big.tile([128, NKT, DH + 1], bf16, tag=f"v{h}", name=f"v{h}")
                for h in range(HPC)]
        zp_sb = [big.tile([128, S + SCC], bf16, tag=f"zp{t}", name=f"zp{t}")
                 for t in range(2)]

        # ---- Phase B: QKV projections ----
        with tc.tile_pool(name="pbqk", bufs=4, space="PSUM") as pbqk, \
             tc.tile_pool(name="pbv", bufs=3, space="PSUM") as pbv:

            def qk_proj(t, w_sb, bcol, dst):
                pss = [pbqk.tile([128, QT], f32, tag="qk",
                                 name=f"qk{t}{st}{w_sb.tensor.name}")
                       for st in range(NQT)]
                for c in range(NDC):
                    for st in range(NQT):
                        nc.tensor.matmul(pss[st], w_sb[:, c, t * 128:(t + 1) * 128],
                                         xt_sb[c][:, st * QT:(st + 1) * QT],
                                         start=(c == 0), stop=(c == NDC - 1))
                for st in range(NQT):
                    nc.vector.tensor_scalar(
                        out=dst[:, t, st * QT:(st + 1) * QT], in0=pss[st],
                        scalar1=bcol[:, t:t + 1], scalar2=None, op0=ALU.add)

            qk_proj(0, wq_sb, bq_sb, qT_sb)
            qk_proj(0, wk_sb, bk_sb, kT_sb)
            for st in range(NKT):
                ps = pbv.tile([128, CW], f32, tag="v", name=f"vv{st}")
                for c in range(NDC):
                    nc.tensor.matmul(ps, xt_sb[c][:, st * 128:(st + 1) * 128],
                                     wv_sb[:, c, :],
                                     start=(c == 0), stop=(c == NDC - 1))
                for h in range(HPC):
                    nc.vector.tensor_copy(out=v_sb[h][:, st, 0:DH],
                                          in_=ps[:, h * DH:(h + 1) * DH])
            for h in range(HPC):
                nc.vector.memset(v_sb[h][:, :, DH:DH + 1], 1.0)
            qk_proj(1, wq_sb, bq_sb, qT_sb)
            qk_proj(1, wk_sb, bk_sb, kT_sb)

        # ---- Phase C: attention; per-qt softmax normalize into payload ----
        with tc.tile_pool(name="psc", bufs=2, space="PSUM") as psc, \
             tc.tile_pool(name="pav", bufs=4, space="PSUM") as pav, \
             tc.tile_pool(name="pexp", bufs=4) as pexp, \
             tc.tile_pool(name="pd", bufs=1) as pd:
            bnst = [pd.tile([64, NQT, 6], f32, tag=f"bn{h}", name=f"bnst{h}")
                    for h in range(HPC)]

            def attn_qt(t, qt):
                h0, h1 = 2 * t, 2 * t + 1
                win = slice(qt * QT, (qt + 1) * QT)
                av0 = pav.tile([DH + 1, QT], f32, tag="av", name=f"av{t}{qt}a")
                av1 = pav.tile([DH + 1, QT], f32, tag="av", name=f"av{t}{qt}b")
                for kt in range(NKT):
                    sps = psc.tile([128, 2 * QT], f32, tag="s", name=f"s{t}{qt}{kt}")
                    for o in range(2):
                        nc.tensor.matmul(
                            sps[:, o * QT:(o + 1) * QT],
                            kT_sb[64 * o:64 * (o + 1), t, kt * 128:(kt + 1) * 128],
                            qT_sb[64 * o:64 * (o + 1), t, qt * QT:(qt + 1) * QT],
                            start=True, stop=True)
                    e_sb = pexp.tile([128, 2 * QT], bf16, tag="e", name=f"e{t}{qt}{kt}")
                    nc.scalar.activation(e_sb, sps, AF.Exp)
                    nc.tensor.matmul(av0, v_sb[h0][:, kt, :], e_sb[:, 0:QT],
                                     start=(kt == 0), stop=(kt == NKT - 1))
                    nc.tensor.matmul(av1, v_sb[h1][:, kt, :], e_sb[:, QT:2 * QT],
                                     start=(kt == 0), stop=(kt == NKT - 1))

                # per-qt softmax normalization; halves go straight to ag_in
                for i, av in enumerate((av0, av1)):
                    h = h0 + i
                    dstg = pd.tile([DH + 1, QT], f32, tag="ds", bufs=4,
                                   name=f"ds{t}{qt}{i}")
                    nc.vector.tensor_copy(out=dstg[DH:DH + 1, :],
                                          in_=av[DH:DH + 1, :])
                    drow = pd.tile([1, QT], f32, tag="dr", bufs=4,
                                   name=f"dr{t}{qt}{i}")
                    nc.sync.dma_start(out=drow, in_=dstg[DH:DH + 1, :])
                    rb = pd.tile([64, QT], f32, tag="rb", bufs=4,
                                 name=f"rb{t}{qt}{i}")
                    nc.gpsimd.partition_broadcast(rb, drow[0:1, :])
                    nc.vector.reciprocal_approx_fast(rb, rb)
                    if i == 0:
                        dst = zp_sb[t][0:64, win]
                        nc.vector.tensor_tensor(out=dst, in0=av[0:DH, :],
                                                in1=rb, op=ALU.mult)
                        nc.vector.bn_stats(out=bnst[h][:, qt, :], in_=dst)
                        nc.sync.dma_start(out=ag_in[t][0:64, win], in_=dst)
                    else:
                        stg = pd.tile([64, QT], bf16, tag="stg", bufs=2,
                                      name=f"stg{t}{qt}")
                        nc.vector.tensor_tensor(out=stg, in0=av[0:DH, :],
                                                in1=rb, op=ALU.mult)
                        nc.vector.bn_stats(out=bnst[h][:, qt, :], in_=stg)
                        nc.gpsimd.dma_start(out=zp_sb[t][64:128, win], in_=stg)
                        nc.sync.dma_start(out=ag_in[t][64:128, win], in_=stg)

            def pair_tail(t):
                # per-head mean/var -> bitcast payload scalar row + collective
                h0, h1 = 2 * t, 2 * t + 1
                msc = pd.tile([1, 4], f32, tag=f"msc{t}", name=f"msc{t}")
                for i, h in enumerate((h0, h1)):
                    mvh = pd.tile([64, 2], f32, tag="mv", bufs=2, name=f"mv{h}")
                    nc.vector.bn_aggr(out=mvh, in_=bnst[h])
                    stk = pd.tile([64, 3], f32, tag="stk", bufs=2, name=f"stk{h}")
                    nc.vector.tensor_add(stk[:, 0:1], mvh[:, 0:1], bv_sb[:, h:h + 1])
                    nc.vector.tensor_copy(stk[:, 1:2], mvh[:, 1:2])
                    nc.vector.tensor_mul(stk[:, 2:3], stk[:, 0:1], stk[:, 0:1])
                    stpt = pav.tile([DH + 1, QT], f32, tag="av",
                                    name=f"stp{h}")
                    stp = stpt[0:1, 0:3]
                    nc.tensor.matmul(stp, onesc, stk, start=True, stop=True)
                    e3 = pd.tile([1, 3], f32, tag="e3", bufs=2, name=f"e3{h}")
                    nc.vector.tensor_scalar(out=e3, in0=stp, scalar1=1.0 / 64.0,
                                            scalar2=None, op0=ALU.mult)
                    m2 = pd.tile([1, 1], f32, tag="m2", bufs=2, name=f"m2{h}")
                    nc.vector.tensor_mul(m2, e3[:, 0:1], e3[:, 0:1])
                    nc.vector.tensor_copy(msc[:, i:i + 1], e3[:, 0:1])
                    vv = pd.tile([1, 1], f32, tag="vv", bufs=2, name=f"vv{h}")
                    nc.vector.tensor_add(vv, e3[:, 1:2], e3[:, 2:3])
                    nc.vector.tensor_tensor(out=msc[:, 2 + i:3 + i], in0=vv,
                                            in1=m2, op=ALU.subtract)
                nc.vector.tensor_copy(out=zp_sb[t][0:1, S:S + 8],
                                      in_=msc[0:1, :].bitcast(bf16))
                nc.sync.dma_start(out=ag_in[t][0:1, S:S + 8],
                                  in_=zp_sb[t][0:1, S:S + 8])
                if with_collective:
                    nc.gpsimd.collective_compute(
                        "AllGather", ALU.bypass,
                        replica_groups=[[0, 1, 2, 3], [4, 5, 6, 7]],
                        ins=[ag_in[t][:].opt()],
                        outs=[ag_out[t][:].opt()],
                    )
                else:
                    for g in range(4):
                        nc.sync.dma_start(out=ag_out[t][g], in_=ag_in[t][:, :])

            for qt in range(NQT):
                attn_qt(0, qt)
            # pair-0 tail is emitted after pair-1 qt0 so its stat matmuls /
            # DVE chain don't stall the PE queue at the pair boundary
            attn_qt(1, 0)
            pair_tail(0)
            for qt in range(1, NQT):
                attn_qt(1, qt)
            pair_tail(1)

        # ---- Phase E: GN fold + column-parallel out-projection ----
        with tc.tile_pool(name="pg", bufs=1) as pg, \
             tc.tile_pool(name="pf", bufs=8, space="PSUM") as pf, \
             tc.tile_pool(name="pystage", bufs=2) as pystage:
            nrm = pg.tile([128, NDC, S], bf16, tag="nrm")
            # gathered scalars per pair: [1, 4(g), 4] f32 = (M_e, M_o, V_e, V_o)
            sc16 = [pg.tile([1, 4, SCC], bf16, tag=f"sc{t}", name=f"sc16{t}")
                    for t in range(2)]
            scf = [sc16[t][:, :, 0:8].bitcast(f32) for t in range(2)]
            # rows [1, 2(o), 4(g), 2(t)]; slice [:,o] streams c=2g+t order
            mrow = pg.tile([1, 2, 4, 2], f32, tag="mrow")
            rrow_ = pg.tile([1, 2, 4, 2], f32, tag="rrowp")
            s2c = pg.tile([128, NDC], f32, tag="s2c")
            m2c = pg.tile([128, NDC], f32, tag="m2c")
            wos = pg.tile([128, NDC, CW], bf16, tag="wos")
            yps = [[pf.tile([128, QT], f32, tag="y", name=f"yp{nt}{st}")
                    for st in range(NQT)] for nt in range(2)]

            def fold_pair(t):
                # vars -> rstd (one ACT sqrt table switch; exp is done by now)
                nc.sync.dma_start(
                    out=sc16[t],
                    in_=ag_out[t][:, 0:1, S:S + SCC].rearrange("g p c -> p g c"))
                va = pg.tile([1, 4, 2], f32, tag="va", bufs=2, name=f"va{t}")
                nc.vector.tensor_copy(va, scf[t][:, :, 2:4])
                sd = pg.tile([1, 4, 2], f32, tag="sd", bufs=2, name=f"sd{t}")
                nc.scalar.activation(sd, va, AF.Sqrt, bias=eps_t)
                rr = pg.tile([1, 4, 2], f32, tag="rrr", bufs=2, name=f"rr{t}")
                nc.vector.reciprocal(rr, sd)
                for o in range(2):
                    nc.vector.tensor_copy(out=mrow[:, o, :, t],
                                          in_=scf[t][:, :, o])
                    nc.vector.tensor_copy(out=rrow_[:, o, :, t],
                                          in_=rr[:, :, o])

            def plane_fill(dst, rows, nm):
                # dst[64o:64(o+1), c] = rows[o, c]: broadcast each half-row
                # to a full-height scratch, then merge at matching offsets
                # (cross-offset engine ops are broken; see pb_probe).
                for o in range(2):
                    pl = pg.tile([128, NDC], f32, tag="plf", bufs=2,
                                 name=f"pl{nm}{o}")
                    nc.gpsimd.partition_broadcast(pl, rows[:, o, :, :])
                    nc.vector.tensor_copy(out=dst[64 * o:64 * (o + 1), :],
                                          in_=pl[64 * o:64 * (o + 1), :])

            def scale_chunks(t):
                plane_fill(s2c, rrow_, f"s{t}")
                for g in range(4):
                    c = 2 * g + t
                    nc.vector.tensor_scalar(out=wos[:, c, :], in0=wo_sb[:, c, :],
                                            scalar1=s2c[:, c:c + 1], scalar2=None,
                                            op0=ALU.mult)

            def outproj_chunks(t, final=False):
                for g in range(4):
                    c = 2 * g + t
                    nc.sync.dma_start(out=nrm[:, c, :], in_=ag_out[t][g, :, 0:S])
                    for nt in range(2):
                        for st in range(NQT):
                            nc.tensor.matmul(
                                yps[nt][st], wos[:, c, nt * 128:(nt + 1) * 128],
                                nrm[:, c, st * QT:(st + 1) * QT],
                                start=(t == 0 and g == 0),
                                stop=(final and g == 3))

            fold_pair(0)
            scale_chunks(0)
            outproj_chunks(0)
            fold_pair(1)
            scale_chunks(1)

            # wsum[p,n] = sum_c wo[p,c,n]*(bv-M)[p,c]*r[p,c]; +bo on row 0.
            # Its ones-rank-1 matmul adds the GN constant + bias to every y.
            plane_fill(m2c, mrow, "m")
            mcs = pg.tile([128, NDC], f32, tag="mcs")
            nc.vector.tensor_tensor(out=mcs, in0=bvg_sb, in1=m2c,
                                    op=ALU.subtract)
            mvec = pg.tile([128, NDC], f32, tag="mvec")
            nc.vector.tensor_mul(mvec, mcs, s2c)
            wsum = pg.tile([128, CW], f32, tag="wsum")
            nc.vector.tensor_scalar(out=wsum, in0=wo_sb[:, 0, :],
                                    scalar1=mvec[:, 0:1], scalar2=None,
                                    op0=ALU.mult)
            for c in range(1, NDC):
                nc.vector.scalar_tensor_tensor(
                    out=wsum, in0=wo_sb[:, c, :], scalar=mvec[:, c:c + 1],
                    in1=wsum, op0=ALU.mult, op1=ALU.add)
            nc.vector.tensor_add(wsum[0:1, :], wsum[0:1, :], bo_sb)
            wsumb = pg.tile([128, CW], bf16, tag="wsumb")
            nc.vector.tensor_copy(wsumb, wsum)

            if debug_taps:
                dbg_sb = pg.tile([128, 2 * NDC + CW], f32, tag="dbg")
                nc.vector.tensor_copy(dbg_sb[:, 0:NDC], s2c)
                nc.vector.tensor_copy(dbg_sb[:, NDC:2 * NDC], m2c)
                nc.vector.tensor_copy(dbg_sb[:, 2 * NDC:], wsum)
                nc.sync.dma_start(out=dbg_d[:, :], in_=dbg_sb)
                for t in range(2):
                    zt = pg.tile([128, S + SCC], f32, tag=f"dz{t}",
                                 name=f"dz{t}")
                    nc.vector.tensor_copy(zt, zp_sb[t])
                    nc.sync.dma_start(out=dbgz_d[t], in_=zt)

            for nt in range(2):
                for st in range(NQT):
                    nc.tensor.matmul(yps[nt][st],
                                     wsumb[:, nt * 128:(nt + 1) * 128],
                                     ones128, start=False, stop=False)
            outproj_chunks(1, final=True)

            for nt in range(2):
                ystage = pystage.tile([128, S], f32, tag="ys", name=f"ys{nt}")
                for st in range(NQT):
                    nc.scalar.activation(ystage[:, st * QT:(st + 1) * QT],
                                         yps[nt][st], AF.Copy)
                    nc.sync.dma_start(out=y_d[nt, :, st * QT:(st + 1) * QT],
                                      in_=ystage[:, st * QT:(st + 1) * QT])

    nc.compile()
    return nc


def _get_nc():
    if "nc" not in _cache:
        _cache["nc"] = _build()
    return _cache["nc"]


def _host_prep(x, Wq, bq, Wk, bk, Wv, bv, Wo, bo, lq1, lk1, lq2, lk2, gn_w, gn_b):
    x = np.asarray(x, np.float32)
    lam = (np.exp((np.asarray(lq1) * np.asarray(lk1)).sum(-1))
           - np.exp((np.asarray(lq2) * np.asarray(lk2)).sum(-1)) + LAMBDA_INIT)
    qscale = (DH ** -0.5) * lam
    Wq_eff = (np.asarray(Wq).reshape(D, H, DH) * qscale[None, :, None]).reshape(D, D)
    bq_eff = (np.asarray(bq).reshape(H, DH) * qscale[:, None]).reshape(D)
    gw = np.asarray(gn_w).reshape(D)
    gb = np.asarray(gn_b).reshape(D)
    Wo_eff = np.asarray(Wo) * gw[:, None]
    bo_eff = np.asarray(bo) + gb @ np.asarray(Wo)

    xT = np.ascontiguousarray(x.transpose(0, 2, 1))  # [B, D, S]
    bf = ml_dtypes.bfloat16

    def bias_cols(bvec, cs):
        # [128, 2] f32: column t = per-partition bias for pair half t
        b = np.asarray(bvec)[cs].reshape(2, 2, DH)  # [t, o, dh]
        out = np.empty((128, 2), np.float32)
        for t in range(2):
            out[:, t] = b[t].reshape(128)
        return np.ascontiguousarray(out)

    in_maps = []
    for c in range(N_CORES):
        b, hg = c // 4, c % 4
        cs = slice(CW * hg, CW * (hg + 1))
        in_maps.append({
            "xt": np.ascontiguousarray(xT[b]).astype(bf),
            "wq": np.ascontiguousarray(Wq_eff[:, cs]).astype(bf),
            "wk": np.ascontiguousarray(np.asarray(Wk)[:, cs]).astype(bf),
            "wv": np.ascontiguousarray(np.asarray(Wv)[:, cs]).astype(bf),
            "wo": np.ascontiguousarray(Wo_eff[:, cs]).astype(bf),
            "bq2": bias_cols(bq_eff, cs),
            "bk2": bias_cols(np.asarray(bk), cs),
            "bv": np.ascontiguousarray(
                np.asarray(bv)[cs].reshape(HPC, DH).T).astype(np.float32),
            "bvf": np.ascontiguousarray(np.asarray(bv)).astype(np.float32),
            "bor": np.ascontiguousarray(
                bo_eff[cs].reshape(1, CW)).astype(np.float32),
        })
    return in_maps


def _host_gather(outs):
    # core c=4b+hg produced output columns [256*hg, 256*(hg+1)) as [2,128,S]
    yT = np.empty((B, D, S), np.float32)
    for b in range(B):
        for hg in range(4):
            q = np.asarray(outs[4 * b + hg]["y"]).reshape(CW, S)
            yT[b, CW * hg:CW * (hg + 1), :] = q
    return np.ascontiguousarray(yT.transpose(0, 2, 1))


def kernel(x, Wq, bq, Wk, bk, Wv, bv, Wo, bo, lq1, lk1, lq2, lk2, gn_w, gn_b):
    from concourse.bass_utils import run_bass_kernel_spmd

    in_maps = _host_prep(x, Wq, bq, Wk, bk, Wv, bv, Wo, bo,
                         lq1, lk1, lq2, lk2, gn_w, gn_b)
    nc = _get_nc()
    res = run_bass_kernel_spmd(nc, in_maps, core_ids=list(range(N_CORES)))
    return _host_gather(res.results)


# revision 17
# speedup vs baseline: 1.0967x; 1.0967x over previous
"""Multi-head differential attention on 8 Trainium2 NeuronCores.

Sharding: core c -> batch c//4, head-group c%4 (4 of 16 heads).
Per core: QKV projection for its heads, k-major attention (scores
transposed; softmax denominators come from a ones-row appended to V via
the AV matmul).  Softmax normalization is pipelined PER QUERY TILE: the
denominator row is reciprocal'd ([1,512] DVE), broadcast across 64
partitions (stride-0 DMA), and one DVE multiply writes the normalized
bf16 z straight into the gather payload; bn_stats runs on the bf16
slices.  Each head-pair's payload (z plus bitcast [mean,var] scalar
columns) is DMA'd per-qt into DRAM and AllGathered within the 4-core
batch group; pair 0's gather hides under pair 1's attention.  GN rstd
(sqrt) is computed on the CONSUMER side after the gathers land, so the
ACT engine does a single exp->sqrt table switch per kernel.  The
column-parallel out-projection accumulates gathered chunks in PSUM,
prefolding the pair-0 chunks while gather #2 is in flight; the GN
affine constant row rides the ACT copy bias.

Host side folds: lambda and softmax scale into Wq/bq; gn_w into Wo;
gb@Wo into bo.  x is pre-transposed per batch and cast to bf16.
"""

import numpy as np
import ml_dtypes

B, S, D, H, DH = 2, 2048, 1024, 16, 64
HPC = 4            # heads per core
CW = HPC * DH      # attention columns per core (256)
EPS = 1e-5
LAMBDA_INIT = 0.8
N_CORES = 8
SCC = 16           # scalar payload columns (bf16); 4 f32 used
NQT = 4            # query tiles of 512
QT = 512
NKT = 16           # key tiles of 128
NDC = 8            # d-chunks of 128

_cache = {}


def _build(with_collective=True, debug_taps=False):
    from contextlib import ExitStack
    import concourse.bass as bass
    from concourse import bacc
    import concourse.tile as tile
    import concourse.mybir as mybir

    f32 = mybir.dt.float32
    bf16 = mybir.dt.bfloat16
    AF = mybir.ActivationFunctionType
    ALU = mybir.AluOpType

    nc = bacc.Bacc("TRN2", target_bir_lowering=False, debug=False,
                   num_devices=N_CORES)

    xt_d = nc.dram_tensor("xt", [D, S], bf16, kind="ExternalInput")
    wq_d = nc.dram_tensor("wq", [D, CW], bf16, kind="ExternalInput")
    wk_d = nc.dram_tensor("wk", [D, CW], bf16, kind="ExternalInput")
    wv_d = nc.dram_tensor("wv", [D, CW], bf16, kind="ExternalInput")
    wo_d = nc.dram_tensor("wo", [D, CW], bf16, kind="ExternalInput")
    bq_d = nc.dram_tensor("bq2", [128, 2], f32, kind="ExternalInput")
    bk_d = nc.dram_tensor("bk2", [128, 2], f32, kind="ExternalInput")
    bv_d = nc.dram_tensor("bv", [64, HPC], f32, kind="ExternalInput")
    bvf_d = nc.dram_tensor("bvf", [D], f32, kind="ExternalInput")
    bo_d = nc.dram_tensor("bor", [1, CW], f32, kind="ExternalInput")
    y_d = nc.dram_tensor("y", [2, 128, S], f32, kind="ExternalOutput")

    if debug_taps:
        dbg_d = nc.dram_tensor("dbg", [128, 2 * NDC + CW], f32,
                               kind="ExternalOutput")
        dbgz_d = nc.dram_tensor("dbgz", [2, 128, S + SCC], f32,
                                kind="ExternalOutput")
    ag_in = [nc.dram_tensor(f"ag_in{t}", [128, S + SCC], bf16)
             for t in range(2)]
    ag_out = [nc.dram_tensor(f"ag_out{t}", [4, 128, S + SCC], bf16)
              for t in range(2)]

    with ExitStack() as ctx:
        tc = ctx.enter_context(tile.TileContext(nc))
        const = ctx.enter_context(tc.tile_pool(name="const", bufs=1))
        big = ctx.enter_context(tc.tile_pool(name="big", bufs=1))

        # ---- inputs, priority order: what the first matmuls need first ----
        wq_sb = const.tile([128, NDC, CW], bf16, tag="wq")
        nc.sync.dma_start(out=wq_sb[:, :, 0:128],
                          in_=wq_d[:, 0:128].rearrange("(c p) n -> p c n", p=128))
        bq_sb = const.tile([128, 2], f32, tag="bq")
        nc.sync.dma_start(out=bq_sb, in_=bq_d[:, :])
        pxt = ctx.enter_context(tc.tile_pool(name="pxt", bufs=1))
        xt_sb = [pxt.tile([128, S], bf16, tag=f"xt{c}", name=f"xt{c}")
                 for c in range(NDC)]
        nc.sync.dma_start(out=xt_sb[0], in_=xt_d[0:128, :])
        wk_sb = const.tile([128, NDC, CW], bf16, tag="wk")
        nc.sync.dma_start(out=wk_sb[:, :, 0:128],
                          in_=wk_d[:, 0:128].rearrange("(c p) n -> p c n", p=128))
        bk_sb = const.tile([128, 2], f32, tag="bk")
        nc.sync.dma_start(out=bk_sb, in_=bk_d[:, :])
        for c in range(1, 4):
            nc.sync.dma_start(out=xt_sb[c], in_=xt_d[c * 128:(c + 1) * 128, :])
        wv_sb = const.tile([128, NDC, CW], bf16, tag="wv")
        nc.sync.dma_start(out=wv_sb, in_=wv_d[:, :].rearrange("(c p) n -> p c n", p=128))
        for c in range(4, NDC):
            nc.sync.dma_start(out=xt_sb[c], in_=xt_d[c * 128:(c + 1) * 128, :])
        nc.sync.dma_start(out=wq_sb[:, :, 128:256],
                          in_=wq_d[:, 128:256].rearrange("(c p) n -> p c n", p=128))
        nc.sync.dma_start(out=wk_sb[:, :, 128:256],
                          in_=wk_d[:, 128:256].rearrange("(c p) n -> p c n", p=128))
        bv_sb = const.tile([64, HPC], f32, tag="bv")
        nc.sync.dma_start(out=bv_sb, in_=bv_d[:, :])
        wo_sb = const.tile([128, NDC, CW], bf16, tag="wo")
        nc.sync.dma_start(out=wo_sb, in_=wo_d[:, :].rearrange("(c p) n -> p c n", p=128))
        bo_sb = const.tile([1, CW], f32, tag="bo")
        nc.sync.dma_start(out=bo_sb, in_=bo_d[:, :])
        bvg_sb = const.tile([128, NDC], f32, tag="bvg")
        nc.sync.dma_start(out=bvg_sb, in_=bvf_d[:].rearrange("(c p) -> p c", p=128))

        onesc = const.tile([64, 1], f32, tag="onesc")
        nc.vector.memset(onesc, 1.0)
        ones128 = const.tile([128, QT], bf16, tag="ones128")
        nc.vector.memset(ones128, 1.0)
        eps_t = const.tile([1, 1], f32, tag="eps")
        nc.vector.memset(eps_t, EPS)

        qT_sb = big.tile([128, 2, S], bf16, tag="qT")   # pair t: head 2t rows 0:64
        kT_sb = big.tile([128, 2, S], bf16, tag="kT")
        v_sb = [big.tile([128, NKT, DH + 1], bf16, tag=f"v{h}", name=f"v{h}")
                for h in range(HPC)]
        zp_sb = [big.tile([128, S + SCC], bf16, tag=f"zp{t}", name=f"zp{t}")
                 for t in range(2)]

        # ---- Phase B: QKV projections ----
        with tc.tile_pool(name="pbqk", bufs=4, space="PSUM") as pbqk, \
             tc.tile_pool(name="pbv", bufs=3, space="PSUM") as pbv:

            def qk_proj(t, w_sb, bcol, dst):
                pss = [pbqk.tile([128, QT], f32, tag="qk",
                                 name=f"qk{t}{st}{w_sb.tensor.name}")
                       for st in range(NQT)]
                for c in range(NDC):
                    for st in range(NQT):
                        nc.tensor.matmul(pss[st], w_sb[:, c, t * 128:(t + 1) * 128],
                                         xt_sb[c][:, st * QT:(st + 1) * QT],
                                         start=(c == 0), stop=(c == NDC - 1))
                for st in range(NQT):
                    nc.vector.tensor_scalar(
                        out=dst[:, t, st * QT:(st + 1) * QT], in0=pss[st],
                        scalar1=bcol[:, t:t + 1], scalar2=None, op0=ALU.add)

            qk_proj(0, wq_sb, bq_sb, qT_sb)
            qk_proj(0, wk_sb, bk_sb, kT_sb)
            for st in range(NKT):
                ps = pbv.tile([128, CW], f32, tag="v", name=f"vv{st}")
                for c in range(NDC):
                    nc.tensor.matmul(ps, xt_sb[c][:, st * 128:(st + 1) * 128],
                                     wv_sb[:, c, :],
                                     start=(c == 0), stop=(c == NDC - 1))
                for h in range(HPC):
                    nc.vector.tensor_copy(out=v_sb[h][:, st, 0:DH],
                                          in_=ps[:, h * DH:(h + 1) * DH])
            for h in range(HPC):
                nc.vector.memset(v_sb[h][:, :, DH:DH + 1], 1.0)
            qk_proj(1, wq_sb, bq_sb, qT_sb)
            qk_proj(1, wk_sb, bk_sb, kT_sb)

        # ---- Phase C: attention; per-qt softmax normalize into payload ----
        with tc.tile_pool(name="psc", bufs=2, space="PSUM") as psc, \
             tc.tile_pool(name="pav", bufs=4, space="PSUM") as pav, \
             tc.tile_pool(name="pexp", bufs=4) as pexp, \
             tc.tile_pool(name="pd", bufs=1) as pd:
            bnst = [pd.tile([64, NQT, 6], f32, tag=f"bn{h}", name=f"bnst{h}")
                    for h in range(HPC)]

            def attn_qt(t, qt, hook=None):
                h0, h1 = 2 * t, 2 * t + 1
                win = slice(qt * QT, (qt + 1) * QT)
                av0 = pav.tile([DH + 1, QT], f32, tag="av", name=f"av{t}{qt}a")
                av1 = pav.tile([DH + 1, QT], f32, tag="av", name=f"av{t}{qt}b")
                for kt in range(NKT):
                    if kt == 3 and hook is not None:
                        hook()
                    sps = psc.tile([128, 2 * QT], f32, tag="s", name=f"s{t}{qt}{kt}")
                    for o in range(2):
                        nc.tensor.matmul(
                            sps[:, o * QT:(o + 1) * QT],
                            kT_sb[64 * o:64 * (o + 1), t, kt * 128:(kt + 1) * 128],
                            qT_sb[64 * o:64 * (o + 1), t, qt * QT:(qt + 1) * QT],
                            start=True, stop=True)
                    e_sb = pexp.tile([128, 2 * QT], bf16, tag="e", name=f"e{t}{qt}{kt}")
                    nc.scalar.activation(e_sb, sps, AF.Exp)
                    nc.tensor.matmul(av0, v_sb[h0][:, kt, :], e_sb[:, 0:QT],
                                     start=(kt == 0), stop=(kt == NKT - 1))
                    nc.tensor.matmul(av1, v_sb[h1][:, kt, :], e_sb[:, QT:2 * QT],
                                     start=(kt == 0), stop=(kt == NKT - 1))

                # per-qt softmax normalization; halves go straight to ag_in
                for i, av in enumerate((av0, av1)):
                    h = h0 + i
                    dstg = pd.tile([DH + 1, QT], f32, tag="ds", bufs=4,
                                   name=f"ds{t}{qt}{i}")
                    nc.vector.tensor_copy(out=dstg[DH:DH + 1, :],
                                          in_=av[DH:DH + 1, :])
                    drow = pd.tile([1, QT], f32, tag="dr", bufs=4,
                                   name=f"dr{t}{qt}{i}")
                    nc.sync.dma_start(out=drow, in_=dstg[DH:DH + 1, :])
                    rb = pd.tile([64, QT], f32, tag="rb", bufs=4,
                                 name=f"rb{t}{qt}{i}")
                    nc.gpsimd.partition_broadcast(rb, drow[0:1, :])
                    nc.vector.reciprocal_approx_fast(rb, rb)
                    if i == 0:
                        dst = zp_sb[t][0:64, win]
                        nc.vector.tensor_tensor(out=dst, in0=av[0:DH, :],
                                                in1=rb, op=ALU.mult)
                        nc.vector.bn_stats(out=bnst[h][:, qt, :], in_=dst)
                        nc.sync.dma_start(out=ag_in[t][0:64, win], in_=dst)
                    else:
                        stg = pd.tile([64, QT], bf16, tag="stg", bufs=2,
                                      name=f"stg{t}{qt}")
                        nc.vector.tensor_tensor(out=stg, in0=av[0:DH, :],
                                                in1=rb, op=ALU.mult)
                        nc.vector.bn_stats(out=bnst[h][:, qt, :], in_=stg)
                        nc.gpsimd.dma_start(out=zp_sb[t][64:128, win], in_=stg)
                        nc.sync.dma_start(out=ag_in[t][64:128, win], in_=stg)

            def pair_tail(t):
                # per-head mean/var -> bitcast payload scalar row + collective
                h0, h1 = 2 * t, 2 * t + 1
                msc = pd.tile([1, 4], f32, tag=f"msc{t}", name=f"msc{t}")
                for i, h in enumerate((h0, h1)):
                    mvh = pd.tile([64, 2], f32, tag="mv", bufs=2, name=f"mv{h}")
                    nc.vector.bn_aggr(out=mvh, in_=bnst[h])
                    stk = pd.tile([64, 3], f32, tag="stk", bufs=2, name=f"stk{h}")
                    nc.vector.tensor_add(stk[:, 0:1], mvh[:, 0:1], bv_sb[:, h:h + 1])
                    nc.vector.tensor_copy(stk[:, 1:2], mvh[:, 1:2])
                    nc.vector.tensor_mul(stk[:, 2:3], stk[:, 0:1], stk[:, 0:1])
                    stpt = pav.tile([DH + 1, QT], f32, tag="av",
                                    name=f"stp{h}")
                    stp = stpt[0:1, 0:3]
                    nc.tensor.matmul(stp, onesc, stk, start=True, stop=True)
                    e3 = pd.tile([1, 3], f32, tag="e3", bufs=2, name=f"e3{h}")
                    nc.vector.tensor_scalar(out=e3, in0=stp, scalar1=1.0 / 64.0,
                                            scalar2=None, op0=ALU.mult)
                    m2 = pd.tile([1, 1], f32, tag="m2", bufs=2, name=f"m2{h}")
                    nc.vector.tensor_mul(m2, e3[:, 0:1], e3[:, 0:1])
                    nc.vector.tensor_copy(msc[:, i:i + 1], e3[:, 0:1])
                    vv = pd.tile([1, 1], f32, tag="vv", bufs=2, name=f"vv{h}")
                    nc.vector.tensor_add(vv, e3[:, 1:2], e3[:, 2:3])
                    nc.vector.tensor_tensor(out=msc[:, 2 + i:3 + i], in0=vv,
                                            in1=m2, op=ALU.subtract)
                nc.vector.tensor_copy(out=zp_sb[t][0:1, S:S + 8],
                                      in_=msc[0:1, :].bitcast(bf16))
                nc.sync.dma_start(out=ag_in[t][0:1, S:S + 8],
                                  in_=zp_sb[t][0:1, S:S + 8])
                if with_collective:
                    nc.gpsimd.collective_compute(
                        "AllGather", ALU.bypass,
                        replica_groups=[[0, 1, 2, 3], [4, 5, 6, 7]],
                        ins=[ag_in[t][:].opt()],
                        outs=[ag_out[t][:].opt()],
                    )
                else:
                    for g in range(4):
                        nc.sync.dma_start(out=ag_out[t][g], in_=ag_in[t][:, :])

            for qt in range(NQT):
                attn_qt(0, qt)
            # pair-0 tail emitted a few kt into pair-1 qt0: the PE has queued
            # work while the tail's DVE stat chain resolves, and the gather-1
            # issue moves by only ~1us
            attn_qt(1, 0, hook=lambda: pair_tail(0))
            for qt in range(1, NQT):
                attn_qt(1, qt)
            pair_tail(1)

        # ---- Phase E: GN fold + column-parallel out-projection ----
        with tc.tile_pool(name="pg", bufs=1) as pg, \
             tc.tile_pool(name="pf", bufs=8, space="PSUM") as pf, \
             tc.tile_pool(name="pystage", bufs=2) as pystage:
            nrm = pg.tile([128, NDC, S], bf16, tag="nrm")
            # gathered scalars per pair: [1, 4(g), 4] f32 = (M_e, M_o, V_e, V_o)
            sc16 = [pg.tile([1, 4, SCC], bf16, tag=f"sc{t}", name=f"sc16{t}")
                    for t in range(2)]
            scf = [sc16[t][:, :, 0:8].bitcast(f32) for t in range(2)]
            # rows [1, 2(o), 4(g), 2(t)]; slice [:,o] streams c=2g+t order
            mrow = pg.tile([1, 2, 4, 2], f32, tag="mrow")
            rrow_ = pg.tile([1, 2, 4, 2], f32, tag="rrowp")
            s2c = pg.tile([128, NDC], f32, tag="s2c")
            m2c = pg.tile([128, NDC], f32, tag="m2c")
            wos = pg.tile([128, NDC, CW], bf16, tag="wos")
            yps = [[pf.tile([128, QT], f32, tag="y", name=f"yp{nt}{st}")
                    for st in range(NQT)] for nt in range(2)]

            def fold_pair(t):
                # vars -> rstd (one ACT sqrt table switch; exp is done by now)
                nc.sync.dma_start(
                    out=sc16[t],
                    in_=ag_out[t][:, 0:1, S:S + SCC].rearrange("g p c -> p g c"))
                va = pg.tile([1, 4, 2], f32, tag="va", bufs=2, name=f"va{t}")
                nc.vector.tensor_copy(va, scf[t][:, :, 2:4])
                sd = pg.tile([1, 4, 2], f32, tag="sd", bufs=2, name=f"sd{t}")
                nc.scalar.activation(sd, va, AF.Sqrt, bias=eps_t)
                rr = pg.tile([1, 4, 2], f32, tag="rrr", bufs=2, name=f"rr{t}")
                nc.vector.reciprocal(rr, sd)
                for o in range(2):
                    nc.vector.tensor_copy(out=mrow[:, o, :, t],
                                          in_=scf[t][:, :, o])
                    nc.vector.tensor_copy(out=rrow_[:, o, :, t],
                                          in_=rr[:, :, o])

            def plane_fill(dst, rows, nm):
                # dst[64o:64(o+1), c] = rows[o, c]: broadcast each half-row
                # to a full-height scratch, then merge at matching offsets
                # (cross-offset engine ops are broken; see pb_probe).
                for o in range(2):
                    pl = pg.tile([128, NDC], f32, tag="plf", bufs=2,
                                 name=f"pl{nm}{o}")
                    nc.gpsimd.partition_broadcast(pl, rows[:, o, :, :])
                    nc.vector.tensor_copy(out=dst[64 * o:64 * (o + 1), :],
                                          in_=pl[64 * o:64 * (o + 1), :])

            def scale_chunks(t):
                plane_fill(s2c, rrow_, f"s{t}")
                for g in range(4):
                    c = 2 * g + t
                    nc.vector.tensor_scalar(out=wos[:, c, :], in0=wo_sb[:, c, :],
                                            scalar1=s2c[:, c:c + 1], scalar2=None,
                                            op0=ALU.mult)

            def outproj_chunks(t, final=False):
                for g in range(4):
                    c = 2 * g + t
                    nc.sync.dma_start(out=nrm[:, c, :], in_=ag_out[t][g, :, 0:S])
                    for nt in range(2):
                        for st in range(NQT):
                            nc.tensor.matmul(
                                yps[nt][st], wos[:, c, nt * 128:(nt + 1) * 128],
                                nrm[:, c, st * QT:(st + 1) * QT],
                                start=(t == 0 and g == 0),
                                stop=(final and g == 3))

            fold_pair(0)
            scale_chunks(0)
            outproj_chunks(0)
            fold_pair(1)
            scale_chunks(1)

            # wsum[p,n] = sum_c wo[p,c,n]*(bv-M)[p,c]*r[p,c]; +bo on row 0.
            # Its ones-rank-1 matmul adds the GN constant + bias to every y.
            plane_fill(m2c, mrow, "m")
            mcs = pg.tile([128, NDC], f32, tag="mcs")
            nc.vector.tensor_tensor(out=mcs, in0=bvg_sb, in1=m2c,
                                    op=ALU.subtract)
            mvec = pg.tile([128, NDC], f32, tag="mvec")
            nc.vector.tensor_mul(mvec, mcs, s2c)
            wsum = pg.tile([128, CW], f32, tag="wsum")
            nc.vector.tensor_scalar(out=wsum, in0=wo_sb[:, 0, :],
                                    scalar1=mvec[:, 0:1], scalar2=None,
                                    op0=ALU.mult)
            for c in range(1, NDC):
                nc.vector.scalar_tensor_tensor(
                    out=wsum, in0=wo_sb[:, c, :], scalar=mvec[:, c:c + 1],
                    in1=wsum, op0=ALU.mult, op1=ALU.add)
            nc.vector.tensor_add(wsum[0:1, :], wsum[0:1, :], bo_sb)
            wsumb = pg.tile([128, CW], bf16, tag="wsumb")
            nc.vector.tensor_copy(wsumb, wsum)

            if debug_taps:
                dbg_sb = pg.tile([128, 2 * NDC + CW], f32, tag="dbg")
                nc.vector.tensor_copy(dbg_sb[:, 0:NDC], s2c)
                nc.vector.tensor_copy(dbg_sb[:, NDC:2 * NDC], m2c)
                nc.vector.tensor_copy(dbg_sb[:, 2 * NDC:], wsum)
                nc.sync.dma_start(out=dbg_d[:, :], in_=dbg_sb)
                for t in range(2):
                    zt = pg.tile([128, S + SCC], f32, tag=f"dz{t}",
                                 name=f"dz{t}")
                    nc.vector.tensor_copy(zt, zp_sb[t])
                    nc.sync.dma_start(out=dbgz_d[t], in_=zt)

            for nt in range(2):
                for st in range(NQT):
                    nc.tensor.matmul(yps[nt][st],
                                     wsumb[:, nt * 128:(nt + 1) * 128],
                                     ones128, start=False, stop=False)
            outproj_chunks(1, final=True)

            for nt in range(2):
                ystage = pystage.tile([128, S], f32, tag="ys", name=f"ys{nt}")
                for st in range(NQT):
                    nc.scalar.activation(ystage[:, st * QT:(st + 1) * QT],
                                         yps[nt][st], AF.Copy)
                    nc.sync.dma_start(out=y_d[nt, :, st * QT:(st + 1) * QT],
                                      in_=ystage[:, st * QT:(st + 1) * QT])

    nc.compile()
    return nc


def _get_nc():
    if "nc" not in _cache:
        _cache["nc"] = _build()
    return _cache["nc"]


def _host_prep(x, Wq, bq, Wk, bk, Wv, bv, Wo, bo, lq1, lk1, lq2, lk2, gn_w, gn_b):
    x = np.asarray(x, np.float32)
    lam = (np.exp((np.asarray(lq1) * np.asarray(lk1)).sum(-1))
           - np.exp((np.asarray(lq2) * np.asarray(lk2)).sum(-1)) + LAMBDA_INIT)
    qscale = (DH ** -0.5) * lam
    Wq_eff = (np.asarray(Wq).reshape(D, H, DH) * qscale[None, :, None]).reshape(D, D)
    bq_eff = (np.asarray(bq).reshape(H, DH) * qscale[:, None]).reshape(D)
    gw = np.asarray(gn_w).reshape(D)
    gb = np.asarray(gn_b).reshape(D)
    Wo_eff = np.asarray(Wo) * gw[:, None]
    bo_eff = np.asarray(bo) + gb @ np.asarray(Wo)

    xT = np.ascontiguousarray(x.transpose(0, 2, 1))  # [B, D, S]
    bf = ml_dtypes.bfloat16

    def bias_cols(bvec, cs):
        # [128, 2] f32: column t = per-partition bias for pair half t
        b = np.asarray(bvec)[cs].reshape(2, 2, DH)  # [t, o, dh]
        out = np.empty((128, 2), np.float32)
        for t in range(2):
            out[:, t] = b[t].reshape(128)
        return np.ascontiguousarray(out)

    in_maps = []
    for c in range(N_CORES):
        b, hg = c // 4, c % 4
        cs = slice(CW * hg, CW * (hg + 1))
        in_maps.append({
            "xt": np.ascontiguousarray(xT[b]).astype(bf),
            "wq": np.ascontiguousarray(Wq_eff[:, cs]).astype(bf),
            "wk": np.ascontiguousarray(np.asarray(Wk)[:, cs]).astype(bf),
            "wv": np.ascontiguousarray(np.asarray(Wv)[:, cs]).astype(bf),
            "wo": np.ascontiguousarray(Wo_eff[:, cs]).astype(bf),
            "bq2": bias_cols(bq_eff, cs),
            "bk2": bias_cols(np.asarray(bk), cs),
            "bv": np.ascontiguousarray(
                np.asarray(bv)[cs].reshape(HPC, DH).T).astype(np.float32),
            "bvf": np.ascontiguousarray(np.asarray(bv)).astype(np.float32),
            "bor": np.ascontiguousarray(
                bo_eff[cs].reshape(1, CW)).astype(np.float32),
        })
    return in_maps


def _host_gather(outs):
    # core c=4b+hg produced output columns [256*hg, 256*(hg+1)) as [2,128,S]
    yT = np.empty((B, D, S), np.float32)
    for b in range(B):
        for hg in range(4):
            q = np.asarray(outs[4 * b + hg]["y"]).reshape(CW, S)
            yT[b, CW * hg:CW * (hg + 1), :] = q
    return np.ascontiguousarray(yT.transpose(0, 2, 1))


def kernel(x, Wq, bq, Wk, bk, Wv, bv, Wo, bo, lq1, lk1, lq2, lk2, gn_w, gn_b):
    from concourse.bass_utils import run_bass_kernel_spmd

    in_maps = _host_prep(x, Wq, bq, Wk, bk, Wv, bv, Wo, bo,
                         lq1, lk1, lq2, lk2, gn_w, gn_b)
    nc = _get_nc()
    res = run_bass_kernel_spmd(nc, in_maps, core_ids=list(range(N_CORES)))
    return _host_gather(res.results)
